# revision 10
# baseline (speedup 1.0000x reference)
"""CenterLoss kernel for Trainium2 (8 NeuronCores, SPMD data-parallel).

Reference semantics: loss = clip(distmat * onehot(labels), 1e-12, 1e12).sum()/B
with distmat[i,j] = ||x_i - c_j||^2.  The one-hot mask keeps only
distmat[i, labels_i]; the B*(C-1) masked-out zeros clip to 1e-12, an exact
constant added on the host.  So each core gathers centers[labels] for its 512
batch rows and computes sum ||x_i - c_i||^2 -- O(B*D) work instead of the
O(B*C*D) distance matrix.

Raw bacc (no TileContext), hand-written semaphores.  Per core:
  sync   : labels DMA first (2KB, sole gather dependency -> completes ~8.6us),
           then the x0 tile; final out DMA of the [128,4] partial sums
  scalar : x1..x3 tile DMAs on the ACT HWDGE ring; 4x Square activation with
           free-axis accumulate into acc
  gpsimd : tiny warmup indirect gather at boot (absorbs the SWDGE worker's
           lazy first-use init inside the fixed NEFF entry phase), then 4x
           128-row indirect gathers from centers -- unfused (plain writes):
           desc-gen is ~25% faster than CCE and transfers pace better, and
           the gathers need only the labels, so they never wait on x
  vector : 4x subtract dbuf = x - c, one per gathered tile
Host: pack labels as int32 [128, 4] per core (lab[p, t] = labels[t*128+p]),
sum the 8 partial tiles in f64, add the clip-floor constant, divide by B.
"""

import numpy as np

BATCH = 4096
FEAT = 512
NUM_CLASSES = 10000
N_CORES = 8
ROWS = BATCH // N_CORES
P = 128
NT = ROWS // P

_CACHE = {}


def _build_nc():
    import concourse.bacc as bacc
    import concourse.bass as bass
    import concourse.mybir as mybir

    # shrink the kernel semaphore range: the NEFF entry emits a per-engine
    # EVENT_SEMAPHORE_RANGE_CLEAR over this whole range (~31ns/sem) and the
    # exit resets it again; we use ~20 sems, not 106
    _orig_range = bass.get_kernel_semaphore_range
    bass.get_kernel_semaphore_range = lambda: range(150, 182)
    try:
        nc = bacc.Bacc("TRN2", target_bir_lowering=False, debug=False)
    finally:
        bass.get_kernel_semaphore_range = _orig_range

    x = nc.dram_tensor("x", [ROWS, FEAT], mybir.dt.float32, kind="ExternalInput")
    labels = nc.dram_tensor("labels", [P, NT], mybir.dt.int32, kind="ExternalInput")
    centers = nc.dram_tensor(
        "centers", [NUM_CLASSES, FEAT], mybir.dt.float32, kind="ExternalInput"
    )
    partial = nc.dram_tensor("partial", [P, NT], mybir.dt.float32, kind="ExternalOutput")

    from contextlib import ExitStack

    with ExitStack() as ctx:
        dbuf = ctx.enter_context(nc.sbuf_tensor([P, NT, FEAT], mybir.dt.float32))
        xbuf = ctx.enter_context(nc.sbuf_tensor([P, NT, FEAT], mybir.dt.float32))
        cbuf = ctx.enter_context(nc.sbuf_tensor([P, NT, FEAT], mybir.dt.float32))
        sq = ctx.enter_context(nc.sbuf_tensor([P, NT, FEAT], mybir.dt.float32))
        lab_sb = ctx.enter_context(nc.sbuf_tensor([P, NT], mybir.dt.int32))
        acc = ctx.enter_context(nc.sbuf_tensor([P, NT], mybir.dt.float32))
        warm = ctx.enter_context(nc.sbuf_tensor([P, 1, FEAT], mybir.dt.float32))
        lsem = ctx.enter_context(nc.semaphore("lsem"))
        xsem = ctx.enter_context(nc.semaphore("xsem"))
        gsems = [ctx.enter_context(nc.semaphore(f"g{t}sem")) for t in range(NT)]
        osem = ctx.enter_context(nc.semaphore("osem"))
        vsem = ctx.enter_context(nc.semaphore("vsem"))
        block = ctx.enter_context(nc.Block())

        @block.sync
        def _(sync):
            sync.dma_start(out=lab_sb[:], in_=labels.ap()).then_inc(lsem, 16)
            sync.dma_start(
                out=xbuf[:, 0, :], in_=x.ap()[0:P, :]
            ).then_inc(xsem, 16)
            sync.wait_ge(osem, NT)
            sync.dma_start(out=partial.ap(), in_=acc[:]).then_inc(osem, 16)
            sync.wait_ge(osem, NT + 16)

        @block.gpsimd
        def _(g):
            # warm up the SWDGE worker before inputs land: lazy first-use
            # init (~2.4us) then overlaps the entry phase. Index 0 comes from
            # the const-0.0 pool (bit pattern 0), synced by the preamble
            # barrier; completion folds into gsem0 (subtract0 waits >= 32).
            zidx = nc.const_aps.aps[(mybir.dt.float32, 0.0)][0:16, 0:1].bitcast(
                mybir.dt.int32
            )
            g.indirect_dma_start(
                out=warm[:16, 0, :64],
                out_offset=None,
                in_=centers.ap(),
                in_offset=bass.IndirectOffsetOnAxis(ap=zidx, axis=0),
            ).then_inc(gsems[0], 16)
            g.wait_ge(lsem, 16)
            for t in range(NT):
                g.indirect_dma_start(
                    out=cbuf[:, t, :],
                    out_offset=None,
                    in_=centers.ap(),
                    in_offset=bass.IndirectOffsetOnAxis(
                        ap=lab_sb[:, t : t + 1], axis=0
                    ),
                ).then_inc(gsems[t], 16)

        @block.vector
        def _(v):
            v.wait_ge(xsem, 16 * NT)
            for t in range(NT):
                v.wait_ge(gsems[t], 32 if t == 0 else 16)
                v.tensor_tensor(
                    out=dbuf[:, t, :],
                    in0=xbuf[:, t, :],
                    in1=cbuf[:, t, :],
                    op=mybir.AluOpType.subtract,
                ).then_inc(vsem, 1)

        @block.scalar
        def _(s):
            for t in range(1, NT):
                s.dma_start(
                    out=xbuf[:, t, :], in_=x.ap()[t * P : (t + 1) * P, :]
                ).then_inc(xsem, 16)
            for t in range(NT):
                s.wait_ge(vsem, t + 1)
                s.activation(
                    out=sq[:, t, :],
                    in_=dbuf[:, t, :],
                    func=mybir.ActivationFunctionType.Square,
                    accum_out=acc[:, t : t + 1],
                ).then_inc(osem, 1)

    nc.compile()
    return nc


def _prepare_in_maps(x, centers, labels):
    x = np.asarray(x, dtype=np.float32)
    x = np.ascontiguousarray(x)
    centers = np.ascontiguousarray(np.asarray(centers, dtype=np.float32))
    lab = np.asarray(labels).astype(np.int32).reshape(N_CORES, NT, P)
    lab = np.ascontiguousarray(lab.transpose(0, 2, 1))
    xs = x.reshape(N_CORES, ROWS, FEAT)
    return [
        {"x": xs[i], "labels": lab[i], "centers": centers} for i in range(N_CORES)
    ]


def kernel(x, centers, labels):
    from concourse.bass_utils import run_bass_kernel_spmd

    if "nc" not in _CACHE:
        _CACHE["nc"] = _build_nc()
    nc = _CACHE["nc"]

    in_maps = _prepare_in_maps(x, centers, labels)
    res = run_bass_kernel_spmd(nc, in_maps, core_ids=list(range(N_CORES)))

    total = np.float64(0.0)
    for r in res.results:
        total += r["partial"].astype(np.float64).sum()
    total += np.float64(BATCH) * (NUM_CLASSES - 1) * 1e-12
    return np.float32(total / BATCH)
